# revision 2
# baseline (speedup 1.0000x reference)
"""BertSelfAttention on 8 Trainium2 NeuronCores (Bass/Tile).

Sharding: data-parallel over batch (B=2) x tensor-parallel over heads
(16 heads -> 4 groups of 4). Core c handles batch c//4, head group c%4,
holding column shards of Wq/Wk/Wv. No collectives.

v3: TE-dense schedule. At sustained PE clock the TensorE row count
(~300k cycles ~ 127us @2.4GHz) is the hard floor; everything else must
stay off the critical path:

  * exp split per score tile: ScalarE does one 512-col half (exact exp),
    DVE does the other via a Schraudolph bit-trick in ONE tensor_scalar:
    i16 = round(s * 0.125*128*log2e + (127-sigma)*128) bitcast as bf16
    (~3% max rel err on half the probability mass -> ~1.1e-2 total rel
    err, inside the 2e-2 gate). Each half ~0.7us, so the per-step exp
    wall drops from 1.1-1.3us to ~0.7us and no longer paces the loop.
  * streamed head: xT arrives block-major ([128, 4 blk, 8 kt, 512]; one
    contiguous 8KB/partition DMA per block) and the m=0 K/Q projections
    run per 512-seq block as soon as its DMA lands; pass (0,0) starts
    ~9us in instead of ~28us. GPSIMD cannot touch PSUM (BIR verifier)
    so evacuations stay on ACT (K-m0, ctx out) and DVE (Q-m0, V, m1).
  * fillers (remaining m0 blocks, V tiles, full-sweep m1 units) are
    deadline-scheduled into the per-step TE slack so the PE never idles
    (idle gaps also drop the PE p-state, halving matmul rate).

Engine budget per core: TE ~127us (bound), DVE ~106us, ACT ~102us.

PSUM (8 banks):
  tag "ssc" 2x[128,1024] (4): score tiles, double buffered
  tag "ctx" 2x[65,512]   (2): ctx+denominator accumulators (hh pair)
  tag "a"   2x[128,512]  (2): m0 blocks / V-proj / m1 sweeps / warmup

Per head the ctx stationary is [V_h | ones] (65 cols): PSUM row 65 of
each ctx tile accumulates the softmax denominators for free. Host
unshards: out[b, :, g*256 + 64h + r] = (ctx_h / sums_h).T
"""

import sys

sys.path.insert(0, "/opt/trn_rl_repo")

import numpy as np

try:
    import ml_dtypes

    _BF16 = ml_dtypes.bfloat16
except ImportError:  # pragma: no cover
    import jax.numpy as jnp

    _BF16 = jnp.bfloat16

import concourse.bass as bass
import concourse.mybir as mybir
import concourse.tile as tile
from concourse import bacc
from concourse import bass_utils as _bass_utils
from concourse.bass_utils import run_bass_kernel_spmd

F32 = mybir.dt.float32
BF16 = mybir.dt.bfloat16
I16 = mybir.dt.int16

HIDDEN = 1024
NUM_HEADS = 16
HEAD = 64
B, S = 2, 2048
N_CORES = 8
GROUPS = 4                      # head groups (tensor parallel)
HG = NUM_HEADS // GROUPS        # heads per group = 4
DG = HG * HEAD                  # 256 cols per group
KT_TILES = HIDDEN // 128        # 8 contraction tiles for projections
ST_TILES = S // 128             # 16 sequence tiles
QC = 512                        # q chunk width (one pass = one chunk)
N_QC = S // QC                  # 4
NBLK = 4                        # xT streaming blocks of 512 seq positions
VAUG = HG * (HEAD + 1)          # 260: [V_h | ones] per head

# Schraudolph fast-exp constants (bf16 exponent domain, minimax sigma).
# es = bitcast_bf16(int16(round(s * EXP_MUL + EXP_ADD))) ~= exp(s / 8)
_LOG2E = 1.4426950408889634
EXP_MUL = 0.125 * 128.0 * _LOG2E
EXP_ADD = (127.0 - 0.04303) * 128.0


def _build_kernel():
    nc = bacc.Bacc("TRN2")

    # xT block-major: xTb[p, b, kt, s] = x[b*512+s, kt*128+p]; each
    # [:, b] slice is 8KB contiguous per partition on both sides.
    xTb = nc.dram_tensor("xTb", [128, NBLK, KT_TILES, QC], BF16,
                         kind="ExternalInput")
    # wqk[p, m, kt, :] = [Wq_m | Wk_m][kt*128+p, :] (partition-major
    # SBUF image; 4KB per-partition DMA segments).
    wqk = nc.dram_tensor(
        "wqk", [128, 2, KT_TILES, DG], BF16, kind="ExternalInput"
    )
    # wv pre-augmented (per head 64 cols + zero col), partition-major.
    wv = nc.dram_tensor(
        "wv", [128, KT_TILES, VAUG], BF16, kind="ExternalInput"
    )
    # per-partition bias cols: bq[0:128], bq[128:], bk[0:128], bk[128:]
    bqk = nc.dram_tensor("bqk", [128, 4], F32, kind="ExternalInput")
    # bv interleaved with 1.0 at each head's ones column [1, 260]
    bv_aug = nc.dram_tensor("bv_aug", [1, VAUG], BF16, kind="ExternalInput")
    out_raw = nc.dram_tensor("out_raw", [VAUG, S], F32, kind="ExternalOutput")

    with tile.TileContext(nc) as tc:
        with (
            tc.tile_pool(name="consts", bufs=1) as consts,
            tc.tile_pool(name="esp", bufs=3) as esp,
            tc.tile_pool(name="outp", bufs=4) as outp,
            tc.tile_pool(name="ps", bufs=2, space="PSUM") as ps,
        ):
            # ---- loads: small consts + wv on the scalar queue (idle
            # until the first exp), the wqk0 -> xT blocks -> wqk1 chain
            # on the SP queue. Each SBUF tile is written by ONE queue.
            bqk_sb = consts.tile([128, 4], F32)
            nc.scalar.dma_start(bqk_sb[:], bqk[:])
            bvaug_sb = consts.tile([1, VAUG], BF16)
            nc.scalar.dma_start(bvaug_sb[:], bv_aug[:])
            wv_sb = consts.tile([128, KT_TILES, VAUG], BF16)
            nc.scalar.dma_start(wv_sb[:], wv[:])
            ones_sb = consts.tile([1, QC], BF16)
            nc.vector.memset(ones_sb[:], 1.0)

            wqk_sb = consts.tile([128, 2, KT_TILES, DG], BF16)
            wqk0_sb = wqk_sb[:, 0]
            wqk1_sb = wqk_sb[:, 1]
            xT_sb = consts.tile([128, NBLK, KT_TILES, QC], BF16)
            nc.sync.dma_start(wqk0_sb[:], wqk[:, 0])
            for b in range(NBLK):
                nc.sync.dma_start(xT_sb[:, b], xTb[:, b])
            nc.sync.dma_start(wqk1_sb[:], wqk[:, 1])

            QT_sb = consts.tile([128, 2, S], BF16)
            KT_sb = consts.tile([128, 2, S], BF16)
            v_sb = consts.tile([128, ST_TILES, VAUG], BF16)

            # ---- m=0 projection for one 512-seq block ----
            # K first (scores need K tiles before Q chunks of later
            # passes). wqk m0 layout per kt: Q cols 0:128, K cols 128:256.
            def m0_block(b, which, warm=False):
                col0 = 0 if which == "q" else 128
                acc = ps.tile([128, QC], F32, tag="a", name=f"m0{which}{b}")
                if warm:
                    # p-state warm-up: hold the PE busy through the DMA
                    # load phase; garbage erased by kt=0's start=True.
                    for _ in range(9):
                        nc.tensor.matmul(
                            acc[:], ones_sb[0:1, 0:128], ones_sb[0:1, :],
                            start=True, stop=True,
                        )
                for kt in range(KT_TILES):
                    nc.tensor.matmul(
                        acc[:],
                        wqk0_sb[:, kt, col0:col0 + 128],
                        xT_sb[:, b, kt, :],
                        start=(kt == 0), stop=(kt == KT_TILES - 1),
                    )
                if which == "k":
                    nc.scalar.activation(
                        KT_sb[:, 0, b * QC:(b + 1) * QC], acc[:],
                        mybir.ActivationFunctionType.Identity,
                        bias=bqk_sb[:, 2:3],
                    )
                else:
                    nc.vector.tensor_scalar_add(
                        QT_sb[:, 0, b * QC:(b + 1) * QC], acc[:],
                        bqk_sb[:, 0:1],
                    )

            def v_proj(st):
                psv = ps.tile([128, QC], F32, tag="a", name="psv")
                blk, sub = st // 4, st % 4
                for kt in range(KT_TILES):
                    nc.tensor.matmul(
                        psv[:, 0:VAUG],
                        xT_sb[:, blk, kt, sub * 128:(sub + 1) * 128],
                        wv_sb[:, kt, :],
                        start=(kt == 0), stop=False,
                    )
                nc.tensor.matmul(
                    psv[:, 0:VAUG], ones_sb[:, 0:128], bvaug_sb[:, :],
                    start=False, stop=True,
                )
                nc.vector.tensor_copy(out=v_sb[:, st, :], in_=psv[:, 0:VAUG])

            # m=1 projection: one full 8-kt sweep + DVE evac per unit.
            def qk_m1(dst_sb, wcol, bcol, sc):
                acc = ps.tile([128, QC], F32, tag="a", name=f"m1_{wcol}_{sc}")
                for kt in range(KT_TILES):
                    nc.tensor.matmul(
                        acc[:],
                        wqk1_sb[:, kt, wcol:wcol + 128],
                        xT_sb[:, sc, kt, :],
                        start=(kt == 0), stop=(kt == KT_TILES - 1),
                    )
                nc.vector.tensor_scalar_add(
                    dst_sb[:, 1, sc * QC:(sc + 1) * QC], acc[:],
                    bqk_sb[:, bcol:bcol + 1],
                )

            # ---- head: warmup + block-0 m0 + first two V tiles ----
            m0_block(0, "k", warm=True)
            m0_block(0, "q")
            v_proj(0)
            v_proj(1)

            # ---- filler queue with per-step deadlines ----
            fillers = []
            deadline = {}

            def add_filler(fn, dl):
                deadline[len(fillers)] = dl
                fillers.append(fn)

            # K-m0 blocks: scores of pass (0,0) hit kt=4b at step 4b.
            add_filler(lambda: m0_block(1, "k"), 1)
            add_filler(lambda: v_proj(2), 2)
            add_filler(lambda: v_proj(3), 3)
            add_filler(lambda: m0_block(2, "k"), 4)
            add_filler(lambda: v_proj(4), 4)
            add_filler(lambda: v_proj(5), 5)
            add_filler(lambda: v_proj(6), 6)
            add_filler(lambda: m0_block(3, "k"), 7)
            for st in range(7, ST_TILES):
                add_filler(lambda st=st: v_proj(st), st)
            # Q-m0 block j feeds pass (0,j) starting at step 16j.
            add_filler(lambda: m0_block(1, "q"), 13)
            add_filler(lambda: m0_block(2, "q"), 22)
            add_filler(lambda: m0_block(3, "q"), 40)
            # m=1 sweeps: K all before pass (1,0) at step 64; Q chunk sc
            # before pass (1,sc) at step 64+16sc.
            for i, dl in enumerate((18, 30, 42, 52)):
                add_filler(lambda s=i: qk_m1(KT_sb, 128, 3, s), dl)
            for i, dl in enumerate((58, 72, 88, 102)):
                add_filler(lambda s=i: qk_m1(QT_sb, 0, 1, s), dl)

            # ---- attention: 8 passes x 16 kt steps, lag-1 pipeline ----
            passes = [(p, j) for p in range(2) for j in range(N_QC)]
            steps = [(pi, kt) for pi in range(len(passes))
                     for kt in range(ST_TILES)]
            n_steps = len(steps)

            es_tiles = {}
            ctx_tiles = {}

            def emit_scores(i):
                pi, kt = steps[i]
                p, j = passes[pi]
                ssc = ps.tile([128, 2 * QC], F32, tag="ssc", name="ssc")
                for hh in range(2):
                    rows = slice(hh * 64, hh * 64 + 64)
                    nc.tensor.matmul(
                        ssc[:, hh * QC:(hh + 1) * QC],
                        KT_sb[rows, p, kt * 128:(kt + 1) * 128],
                        QT_sb[rows, p, j * QC:(j + 1) * QC],
                        start=True, stop=True,
                    )
                es = esp.tile([128, 2 * QC], BF16, tag="es", name="es")
                hh_act = i % 2          # exact-exp half alternates
                hh_dve = 1 - hh_act
                sa = slice(hh_act * QC, (hh_act + 1) * QC)
                sd = slice(hh_dve * QC, (hh_dve + 1) * QC)
                nc.scalar.activation(
                    es[:, sa], ssc[:, sa],
                    mybir.ActivationFunctionType.Exp, scale=0.125,
                )
                nc.vector.tensor_scalar(
                    es[:, sd].bitcast(I16), ssc[:, sd],
                    EXP_MUL, EXP_ADD,
                    mybir.AluOpType.mult, mybir.AluOpType.add,
                )
                es_tiles[i] = es

            def emit_ctx(i):
                pi, kt = steps[i]
                p, j = passes[pi]
                if kt == 0:
                    for hh in range(2):
                        ctx_tiles[(pi, hh)] = ps.tile(
                            [65, QC], F32, tag="ctx", name="ctx"
                        )
                es = es_tiles.pop(i)
                for hh in range(2):
                    h = 2 * p + hh
                    nc.tensor.matmul(
                        ctx_tiles[(pi, hh)][:],
                        v_sb[:, kt, h * 65:(h + 1) * 65],
                        es[:, hh * QC:(hh + 1) * QC],
                        start=(kt == 0), stop=(kt == ST_TILES - 1),
                    )
                if kt == ST_TILES - 1:
                    for hh in range(2):
                        h = 2 * p + hh
                        ctx_sb = outp.tile([65, QC], F32, tag="o",
                                           name="ctx_sb")
                        nc.scalar.copy(out=ctx_sb[:],
                                       in_=ctx_tiles.pop((pi, hh))[:])
                        nc.sync.dma_start(
                            out_raw[h * 65:(h + 1) * 65,
                                    j * QC:(j + 1) * QC],
                            ctx_sb[:],
                        )

            fq = list(range(len(fillers)))
            for i in range(n_steps):
                emit_scores(i)
                if i > 0:
                    emit_ctx(i - 1)
                while fq and deadline[fq[0]] <= i:
                    fillers[fq.pop(0)]()
            emit_ctx(n_steps - 1)
            while fq:
                fillers[fq.pop(0)]()
    nc.compile()
    return nc


_NC_CACHE = None


def _get_nc():
    global _NC_CACHE
    if _NC_CACHE is None:
        _NC_CACHE = _build_kernel()
    return _NC_CACHE


def _prep_core_inputs(hidden_states, Wq, bq, Wk, bk, Wv, bv):
    """Host-side sharding: returns list of 8 in_maps (bf16 pre-cast)."""
    # xT [1024, 2048] -> block-major [128 p, 4 blk, 8 kt, 512 s]
    xTbs = [
        np.ascontiguousarray(
            hidden_states[b].T.reshape(KT_TILES, 128, NBLK, QC)
            .transpose(1, 2, 0, 3)
        ).astype(_BF16)
        for b in range(B)
    ]
    in_maps = []
    for c in range(N_CORES):
        b, g = divmod(c, GROUPS)
        cs = slice(g * DG, (g + 1) * DG)
        wq_g = Wq[:, cs]
        wk_g = Wk[:, cs]
        wv_g = Wv[:, cs]
        bq_g, bk_g, bv_g = bq[cs], bk[cs], bv[cs]

        wv_aug = np.zeros((HIDDEN, VAUG), dtype=np.float32)
        bv_aug = np.zeros((1, VAUG), dtype=np.float32)
        for h in range(HG):
            wv_aug[:, h * 65:h * 65 + 64] = wv_g[:, h * 64:(h + 1) * 64]
            bv_aug[0, h * 65:h * 65 + 64] = bv_g[h * 64:(h + 1) * 64]
            bv_aug[0, h * 65 + 64] = 1.0

        bqk = np.stack(
            [bq_g[:128], bq_g[128:], bk_g[:128], bk_g[128:]], axis=1
        ).astype(np.float32)

        in_maps.append(
            {
                "xTb": xTbs[b],
                # partition-major SBUF image [128, 2, 8, 256]
                "wqk": np.ascontiguousarray(
                    np.stack([
                        np.concatenate(
                            [wq_g[:, m * 128:(m + 1) * 128],
                             wk_g[:, m * 128:(m + 1) * 128]], 1
                        ).reshape(KT_TILES, 128, DG).transpose(1, 0, 2)
                        for m in range(2)
                    ], axis=1)
                ).astype(_BF16),
                # partition-major SBUF image [128, 8, 260]
                "wv": np.ascontiguousarray(
                    wv_aug.reshape(KT_TILES, 128, VAUG).transpose(1, 0, 2)
                ).astype(_BF16),
                "bqk": np.ascontiguousarray(bqk),
                "bv_aug": bv_aug.astype(_BF16),
            }
        )
    return in_maps


def _unshard(results):
    out = np.empty((B, S, HIDDEN), dtype=np.float32)
    for c in range(N_CORES):
        b, g = divmod(c, GROUPS)
        raw = results[c]["out_raw"]  # [260, 2048]
        for h in range(HG):
            ctx = raw[h * 65:h * 65 + 64]          # [64, S]
            sums = raw[h * 65 + 64]                # [S]
            col0 = g * DG + h * HEAD
            out[b, :, col0:col0 + HEAD] = (ctx / sums).T
    return out


def kernel(**inputs):
    inputs = {k: np.asarray(v, dtype=np.float32) for k, v in inputs.items()}
    nc = _get_nc()
    in_maps = _prep_core_inputs(**inputs)
    res = run_bass_kernel_spmd(nc, in_maps, core_ids=list(range(N_CORES)))
    return _unshard(res.results)


if __name__ == "__main__":
    rng = np.random.default_rng(0)
    scale = 1.0 / np.sqrt(HIDDEN)
    ins = {
        "hidden_states": rng.standard_normal((B, S, HIDDEN), dtype=np.float32),
        "Wq": rng.standard_normal((HIDDEN, HIDDEN), dtype=np.float32) * scale,
        "bq": rng.standard_normal(HIDDEN, dtype=np.float32) * 0.01,
        "Wk": rng.standard_normal((HIDDEN, HIDDEN), dtype=np.float32) * scale,
        "bk": rng.standard_normal(HIDDEN, dtype=np.float32) * 0.01,
        "Wv": rng.standard_normal((HIDDEN, HIDDEN), dtype=np.float32) * scale,
        "bv": rng.standard_normal(HIDDEN, dtype=np.float32) * 0.01,
    }
    out = kernel(**ins)

    def ref(x, Wq, bq, Wk, bk, Wv, bv):
        q = (x @ Wq + bq).reshape(B, S, NUM_HEADS, HEAD).transpose(0, 2, 1, 3)
        k = (x @ Wk + bk).reshape(B, S, NUM_HEADS, HEAD).transpose(0, 2, 1, 3)
        v = (x @ Wv + bv).reshape(B, S, NUM_HEADS, HEAD).transpose(0, 2, 1, 3)
        s = np.einsum("bhqd,bhkd->bhqk", q, k) / np.sqrt(HEAD)
        s = s - s.max(-1, keepdims=True)
        p = np.exp(s)
        p /= p.sum(-1, keepdims=True)
        c = np.einsum("bhqk,bhkd->bhqd", p, v)
        return c.transpose(0, 2, 1, 3).reshape(B, S, HIDDEN)

    exp = ref(
        ins["hidden_states"].astype(np.float64),
        ins["Wq"].astype(np.float64), ins["bq"].astype(np.float64),
        ins["Wk"].astype(np.float64), ins["bk"].astype(np.float64),
        ins["Wv"].astype(np.float64), ins["bv"].astype(np.float64),
    )
    print("L2 rel err:", np.linalg.norm(out - exp) / np.linalg.norm(exp))
    print("max abs err:", np.abs(out - exp).max())
